# revision 19
# baseline (speedup 1.0000x reference)
"""BinaryLinear kernel for Trainium2 (8 NeuronCores, SPMD).

Computes y = x @ sign(W)^T + sign(b) with x:[8192,4096] f32,
W:[4096,4096] f32, b:[4096] f32.

Sharding: 2-way over tokens x 4-way over out_features (8 cores).
Per core: x_shard [4096, 4096], W_shard [1024, 4096], b_shard [1024]
-> y_shard [4096, 1024]. No collectives; host shards/concats.

Math strategy: sign(W) is exactly representable in bf16 (+-1), and a
SINGLE bf16 pass (y = bf16(x) @ sW^T accumulated in f32 PSUM) gives
~1.2e-3 max-metric relative error -- well under the 2e-2 tolerance.
(The original version used a hi/lo split for ~2e-6 at 2x the PE
work; dropping the lo pass took 1029us -> 702us HW.)

Structure per core:
  - Phase 0: sign(W)^T built resident in SBUF (8MB bf16) -- W tiles
    loaded in waves of 4 with transfers alternated across the
    ACT-HWDGE and SWDGE paths (parallel transfers), ACT Sign, then one
    batched [128, 4096] -> [128, 32, 128] xbar transpose per tile on
    the SP queue. Bias is broadcast-loaded (stride-0 DMA) and signed.
  - Phase 1 (per 128-token tile): SWDGE x load, DVE bf16 cast, one
    xbar transpose into [k, t] layout, 64 MMs into 2 PSUM banks
    (one sweep per 512-wide out group), DVE bias-add eviction,
    SWDGE store.

Variants measured this session (all correct, rel err 1.2e-3):
  v1 (this file) 702552 ns; v5 (+depth-2 prefetch) 717437;
  v3 (pair transposes) 739102; v6 (ACT casts) 743134;
  v7 (og-progressive phase 0) 741912; v2 (SWDGE cast-loads) 776689;
  v4 (shared-stationary og pairs) 766735. The kernel is SDMA-bound
  (DMA union busy 524-553us in every variant: x loads 180us + x
  transposes 166us + W prep 87us + y stores 45us); copies and xbar
  transposes are strictly additive on the 16 SDMA engines, and the
  warm MM cadence varies 215-259ns/MM with chip power state, so the
  depth-1 v1 schedule measured best.

Hardware constraints baked into this structure (learned from NTFF
traces and device crashes):
  - A DMA transpose occupies all 16 DMA engines: it is mutually
    exclusive with copy DMAs and pays a ~10us drain when copies are in
    flight. Keep the SP queue transposes-only and serialize phase 0
    cleanly; overlapping x traffic with W prep measures WORSE.
  - Concurrent transposes issued from two HWDGE queues, or matmuls
    racing a transpose into the same SBUF tile, crash the device
    (NRT_EXEC_UNIT_UNRECOVERABLE).
"""

import sys

sys.path.insert(0, "/opt/trn_rl_repo")

import numpy as np

import concourse.bass as bass  # noqa: F401
import concourse.mybir as mybir
from concourse import bacc, tile
from concourse.bass_utils import run_bass_kernel_spmd

TOKENS, IN, OUT = 8192, 4096, 4096
N_CORES = 8
T_SPLIT, O_SPLIT = 2, 4
T_CORE, O_CORE = TOKENS // T_SPLIT, OUT // O_SPLIT

P = 128
FREE = 512  # matmul moving free dim / psum bank width (f32)

F32 = mybir.dt.float32
BF16 = mybir.dt.bfloat16


def emit(nc, tc, x_d, w_d, b_d, y_d, t_core, in_dim, o_core):
    """Emit the per-core program. x_d [t_core, in], w_d [o_core, in],
    b_d [1, o_core], y_d [t_core, o_core]."""
    KS = in_dim // P  # number of 128-wide k slabs
    TT = t_core // P  # token tiles
    OG = o_core // FREE  # 512-wide out groups
    OT = o_core // P  # 128-row tiles of W

    WARM = 0  # tiles that run progressive 128-wide out sweeps (see v9)

    from contextlib import ExitStack

    with ExitStack() as ctx:
        const = ctx.enter_context(tc.tile_pool(name="const", bufs=1))
        # Resident sign(W)^T: [128 k-part, KS slabs, o_core] bf16
        swt = const.tile([P, KS, o_core], BF16)
        bias_bc = const.tile([P, o_core], F32)

        # ---- Phase 0: weights + bias prep ----
        # Waves of 4 full W tiles: loads on ACT HWDGE, signs on ACT,
        # xbar transposes on the SP queue (transposes only).
        with tc.tile_pool(name="wload", bufs=4) as wpool:
            braw = wpool.tile([P, o_core], F32, name="braw", bufs=1)
            nc.gpsimd.dma_start(braw, b_d.to_broadcast([P, o_core]))
            nc.scalar.sign(bias_bc, braw)
            U16 = mybir.dt.uint16
            wfs = []
            for ot in range(OT):
                wf = wpool.tile([P, in_dim], F32, name="wf")
                eng = nc.scalar if ot % 2 == 0 else nc.gpsimd
                eng.dma_start(wf, w_d[ot * P : (ot + 1) * P, :])
                wfs.append(wf)
            for ot, wf in enumerate(wfs):
                ws = wpool.tile([P, in_dim], BF16, name="ws")
                if ot % 2 == 0:
                    nc.scalar.sign(ws, wf)  # +-1 bf16 on ACT
                else:
                    # DVE bit trick on the f32 high halfwords:
                    # (hi16 & 0x8000) | 0x3F80 == +-1.0 bf16
                    nc.vector.tensor_scalar(
                        out=ws.bitcast(U16),
                        in0=wf.bitcast(U16)[:, 1::2],
                        scalar1=0x8000,
                        scalar2=0x3F80,
                        op0=mybir.AluOpType.bitwise_and,
                        op1=mybir.AluOpType.bitwise_or,
                    )
                # [128 o, in] -> [128 k, KS, 128 o]
                nc.sync.dma_start_transpose(
                    swt[:, :, ot * P : (ot + 1) * P], ws
                )

        # ---- Phase 1 ----
        with (
            tc.tile_pool(name="xload", bufs=2) as xpool,
            tc.tile_pool(name="hilo", bufs=2) as hpool,
            tc.tile_pool(name="xt", bufs=3) as tpool,
            tc.tile_pool(name="psum", bufs=8, space="PSUM") as psum,
            tc.tile_pool(name="yout", bufs=3) as opool,
        ):

            def prep_tile(tt):
                """x f32 load -> bf16 cast (DVE) -> xbar transpose."""
                trow = slice(tt * P, (tt + 1) * P)
                xf = xpool.tile([P, in_dim], F32, name="xf")
                nc.gpsimd.dma_start(xf, x_d[trow, :])
                xhi = hpool.tile([P, in_dim], BF16, name="xhi")
                nc.vector.tensor_copy(out=xhi, in_=xf)
                xhiT = tpool.tile([P, KS, P], BF16, name="xhiT")
                nc.sync.dma_start_transpose(xhiT, xhi)
                return (xhiT,)

            def sweep(ps, xhiT, ocol, width):
                for ks in range(KS):
                    nc.tensor.matmul(
                        ps[:, :width], xhiT[:, ks, :], swt[:, ks, ocol],
                        start=(ks == 0), stop=(ks == KS - 1),
                    )

            def mm_tile(tt, xhiT, owidth):
                """matmul sweeps in owidth-wide out groups + bias evict."""
                trow = slice(tt * P, (tt + 1) * P)
                yo = opool.tile([P, o_core], F32, name="yo")
                for og in range(o_core // owidth):
                    ocol = slice(og * owidth, (og + 1) * owidth)
                    ps = psum.tile([P, FREE], F32, name="ps")
                    sweep(ps, xhiT, ocol, owidth)
                    nc.vector.tensor_tensor(
                        out=yo[:, ocol], in0=ps[:, :owidth],
                        in1=bias_bc[:, ocol], op=mybir.AluOpType.add,
                    )
                # y stores ride the ACT HWDGE ring: on the gpsimd ring
                # the store (which waits tile tt's evictions) queues
                # ahead of load(tt+3), coupling x loads to PE progress
                nc.scalar.dma_start(y_d[trow, :], yo)

            # warmup tiles chase W readiness with 128-wide out groups
            prev = prep_tile(0)
            for tt in range(TT):
                if tt + 1 < TT:
                    nxt = prep_tile(tt + 1)
                mm_tile(tt, *prev, P if tt < WARM else FREE)
                if tt + 1 < TT:
                    prev = nxt


def build(t_core=T_CORE, in_dim=IN, o_core=O_CORE):
    nc = bacc.Bacc("TRN2", target_bir_lowering=False, debug=False)
    x_d = nc.dram_tensor("x", [t_core, in_dim], F32, kind="ExternalInput")
    w_d = nc.dram_tensor("w", [o_core, in_dim], F32, kind="ExternalInput")
    b_d = nc.dram_tensor("b", [1, o_core], F32, kind="ExternalInput")
    y_d = nc.dram_tensor("y", [t_core, o_core], F32, kind="ExternalOutput")
    with tile.TileContext(nc) as tc:
        emit(nc, tc, x_d.ap(), w_d.ap(), b_d.ap(), y_d.ap(), t_core, in_dim, o_core)
    nc.compile()
    return nc


_nc_cache = None


def kernel(x: np.ndarray, weight: np.ndarray, bias: np.ndarray, **run_kwargs):
    global _nc_cache
    if _nc_cache is None:
        _nc_cache = build()
    nc = _nc_cache

    x = np.ascontiguousarray(x, dtype=np.float32)
    weight = np.ascontiguousarray(weight, dtype=np.float32)
    bias = np.ascontiguousarray(bias, dtype=np.float32)

    in_maps = []
    for c in range(N_CORES):
        th, oq = divmod(c, O_SPLIT)
        in_maps.append(
            {
                "x": x[th * T_CORE : (th + 1) * T_CORE],
                "w": weight[oq * O_CORE : (oq + 1) * O_CORE],
                "b": bias[oq * O_CORE : (oq + 1) * O_CORE].reshape(1, O_CORE),
            }
        )
    res = run_bass_kernel_spmd(nc, in_maps, core_ids=list(range(N_CORES)), **run_kwargs)
    y = np.empty((TOKENS, OUT), dtype=np.float32)
    for c in range(N_CORES):
        th, oq = divmod(c, O_SPLIT)
        y[th * T_CORE : (th + 1) * T_CORE, oq * O_CORE : (oq + 1) * O_CORE] = (
            res.results[c]["y"]
        )
    kernel.last_results = res
    return y



# revision 24
# speedup vs baseline: 1.1058x; 1.1058x over previous
"""BinaryLinear kernel for Trainium2 (8 NeuronCores, SPMD). v9.

y = x @ sign(W)^T + sign(b); x[8192,4096] W[4096,4096] b[4096] f32.
Sharding: tokens 2-way x out_features 4-way -> per core
x[4096,4096] W[1024,4096] b[1024] -> y[4096,1024].

Single bf16 pass (~1.2e-3 max-metric rel err vs 2e-2 tolerance).

v9: W^T is built on the (otherwise idle) TensorEngine during phase 0
via is_transpose matmuls against an identity, 128x128 per shot, with
PSUM->swt evictions alternating DVE/ACT. This removes the eight 1MB
xbar W transposes (~42us of serial SDMA time) from phase 0; the xbar
ring then serves x tiles 0-3 during phase 0, so the steady pipeline
starts primed. y stores ride the ACT HWDGE ring (v8: keeps the
gpsimd ring loads decoupled from PE progress).

Known hardware behavior baked in:
  - Copies and xbar transposes are strictly additive on the 16 SDMA
    engines; phase-0 time ~= loads + transposes unless transposes
    move off the SDMA path entirely (this version).
  - DMA union busy was the 524-553us invariant across v1-v8 at
    ~143MB moved; this drops it to ~500us.
  - Run-to-run clock state (2.0 vs 2.4 GHz PE) swings totals ~6%.
"""

import sys

sys.path.insert(0, "/opt/trn_rl_repo")

import numpy as np

import concourse.bass as bass  # noqa: F401
import concourse.mybir as mybir
from concourse import bacc, tile
from concourse.bass_utils import run_bass_kernel_spmd
from concourse.masks import make_identity

TOKENS, IN, OUT = 8192, 4096, 4096
N_CORES = 8
T_SPLIT, O_SPLIT = 2, 4
T_CORE, O_CORE = TOKENS // T_SPLIT, OUT // O_SPLIT

P = 128
FREE = 512

F32 = mybir.dt.float32
BF16 = mybir.dt.bfloat16
U16 = mybir.dt.uint16


def emit(nc, tc, x_d, w_d, b_d, y_d, t_core, in_dim, o_core):
    KS = in_dim // P
    TT = t_core // P
    OT = o_core // P

    from contextlib import ExitStack

    with ExitStack() as ctx:
        const = ctx.enter_context(tc.tile_pool(name="const", bufs=1))
        swt = const.tile([P, KS, o_core], BF16)
        bias_bc = const.tile([P, o_core], F32)
        ident = const.tile([P, P], BF16)
        make_identity(nc, ident)

        xpool = ctx.enter_context(tc.tile_pool(name="xload", bufs=2))
        hpool = ctx.enter_context(tc.tile_pool(name="hilo", bufs=1))
        tpool = ctx.enter_context(tc.tile_pool(name="xt", bufs=4))
        psum = ctx.enter_context(tc.tile_pool(name="psum", bufs=6, space="PSUM"))
        opool = ctx.enter_context(tc.tile_pool(name="yout", bufs=3))

        def prep_tile(tt):
            """x f32 load -> bf16 cast (DVE) -> xbar transpose."""
            trow = slice(tt * P, (tt + 1) * P)
            xf = xpool.tile([P, in_dim], F32, name="xf")
            nc.gpsimd.dma_start(xf, x_d[trow, :])
            xhi = hpool.tile([P, in_dim], BF16, name="xhi")
            nc.vector.tensor_copy(out=xhi, in_=xf)
            xhiT = tpool.tile([P, KS, P], BF16, name="xhiT")
            nc.sync.dma_start_transpose(xhiT, xhi)
            return xhiT

        with tc.tile_pool(name="wload", bufs=1) as wpool:
            braw = wpool.tile([P, o_core], F32, name="braw", bufs=1)
            nc.gpsimd.dma_start(braw, b_d.to_broadcast([P, o_core]))
            nc.scalar.sign(bias_bc, braw)

            # x tiles 0-3 prepped during phase 0: the xbar ring has no
            # W transposes to fight, only the W/x load copies (drains)
            pend = {}
            for ot in range(OT):
                wf = wpool.tile([P, in_dim], F32, name="wf", bufs=2)
                nc.scalar.dma_start(wf, w_d[ot * P : (ot + 1) * P, :])
                ws = wpool.tile([P, in_dim], BF16, name="ws", bufs=2)
                nc.vector.tensor_scalar(
                    out=ws.bitcast(U16),
                    in0=wf.bitcast(U16)[:, 1::2],
                    scalar1=0x8000,
                    scalar2=0x3F80,
                    op0=mybir.AluOpType.bitwise_and,
                    op1=mybir.AluOpType.bitwise_or,
                )
                if ot < 4:
                    pend[ot] = prep_tile(ot)
                # PE transposes, 8 slabs batched into one full PSUM
                # bank (one eviction per bank amortizes the ~2us
                # cross-engine semaphore round trip), evictions
                # alternating DVE/ACT
                for kg in range(KS // 8):
                    psT = psum.tile([P, 8, P], BF16, name="psT", bufs=2)
                    for j in range(8):
                        ks = kg * 8 + j
                        nc.tensor.transpose(
                            psT[:, j, :], ws[:, ks * P : (ks + 1) * P],
                            ident,
                        )
                    dst = swt[:, kg * 8 : (kg + 1) * 8, ot * P : (ot + 1) * P]
                    if kg % 2 == 0:
                        nc.vector.tensor_copy(out=dst, in_=psT)
                    else:
                        nc.scalar.copy(dst, psT)

        # ---- Phase 1 ----
        def mm_tile(tt, xhiT):
            trow = slice(tt * P, (tt + 1) * P)
            yo = opool.tile([P, o_core], F32, name="yo")
            for og in range(o_core // FREE):
                ocol = slice(og * FREE, (og + 1) * FREE)
                ps = psum.tile([P, FREE], F32, name="ps", bufs=6)
                for ks in range(KS):
                    nc.tensor.matmul(
                        ps, xhiT[:, ks, :], swt[:, ks, ocol],
                        start=(ks == 0), stop=(ks == KS - 1),
                    )
                nc.vector.tensor_tensor(
                    out=yo[:, ocol], in0=ps,
                    in1=bias_bc[:, ocol], op=mybir.AluOpType.add,
                )
            nc.scalar.dma_start(y_d[trow, :], yo)

        for tt in range(TT):
            mm_tile(tt, pend.pop(tt))
            if tt + 4 < TT:
                pend[tt + 4] = prep_tile(tt + 4)


def build(t_core=T_CORE, in_dim=IN, o_core=O_CORE):
    nc = bacc.Bacc("TRN2", target_bir_lowering=False, debug=False)
    x_d = nc.dram_tensor("x", [t_core, in_dim], F32, kind="ExternalInput")
    w_d = nc.dram_tensor("w", [o_core, in_dim], F32, kind="ExternalInput")
    b_d = nc.dram_tensor("b", [1, o_core], F32, kind="ExternalInput")
    y_d = nc.dram_tensor("y", [t_core, o_core], F32, kind="ExternalOutput")
    with tile.TileContext(nc) as tc:
        emit(nc, tc, x_d.ap(), w_d.ap(), b_d.ap(), y_d.ap(), t_core, in_dim, o_core)
    nc.compile()
    return nc


_nc_cache = None


def kernel(x: np.ndarray, weight: np.ndarray, bias: np.ndarray, **run_kwargs):
    global _nc_cache
    if _nc_cache is None:
        _nc_cache = build()
    nc = _nc_cache

    x = np.ascontiguousarray(x, dtype=np.float32)
    weight = np.ascontiguousarray(weight, dtype=np.float32)
    bias = np.ascontiguousarray(bias, dtype=np.float32)

    in_maps = []
    for c in range(N_CORES):
        th, oq = divmod(c, O_SPLIT)
        in_maps.append(
            {
                "x": x[th * T_CORE : (th + 1) * T_CORE],
                "w": weight[oq * O_CORE : (oq + 1) * O_CORE],
                "b": bias[oq * O_CORE : (oq + 1) * O_CORE].reshape(1, O_CORE),
            }
        )
    res = run_bass_kernel_spmd(nc, in_maps, core_ids=list(range(N_CORES)), **run_kwargs)
    y = np.empty((TOKENS, OUT), dtype=np.float32)
    for c in range(N_CORES):
        th, oq = divmod(c, O_SPLIT)
        y[th * T_CORE : (th + 1) * T_CORE, oq * O_CORE : (oq + 1) * O_CORE] = (
            res.results[c]["y"]
        )
    kernel.last_results = res
    return y
